# Initial kernel scaffold
#
"""Batched 2x2 complex Hermitian Cholesky on 8 Trainium2 NeuronCores.

Math per matrix (inputs r = real_part[m], s = imag_part[m], both 2x2 f32):
    a   = r00 + 2                      (diag of Hermitian + M*I, M=2)
    c   = r11 + 2
    br  = (r01 + r10) / 2              (real part of A[0,1])
    bi  = (s01 - s10) / 2              (imag part of A[0,1])
    l11 = sqrt(a)
    l21 = conj(b) / l11 = (br - i*bi) / sqrt(a)
    l22 = sqrt(c - |l21|^2)
Output (complex64, upper triangle zero):
    [[l11, 0], [l21, l22]]

Layout: each matrix is 4 contiguous f32 in the input, 8 contiguous f32
(4 complex) in the output.  Per core we process NCHUNK chunks of
128 partitions x KC matrices; all HBM traffic is fully contiguous, the
de-interleave (stride 4/8) happens in SBUF via strided access patterns.
"""

import numpy as np

import concourse.bass as bass
import concourse.mybir as mybir
from concourse import tile
from concourse.bass_utils import run_bass_kernel_spmd

B = 4194304
NCORE = 8
BC = B // NCORE            # 524288 matrices per core
KC = 512                   # matrices per partition per chunk
NCHUNK = BC // (128 * KC)  # 8
F_IN = 4 * KC              # f32 per partition per chunk (input tensors)
F_OUT = 8 * KC             # f32 per partition per chunk (output)
NBUF_OUT = 3               # rotation depth of persistent output buffers

_CACHE = {}


def _build_nc():
    if "nc" in _CACHE:
        return _CACHE["nc"]
    f32 = mybir.dt.float32
    AF = mybir.ActivationFunctionType
    OP = mybir.AluOpType

    nc = bass.Bass("TRN2", target_bir_lowering=False, debug=False)
    xr = nc.dram_tensor("xr", [NCHUNK, 128, F_IN], f32, kind="ExternalInput").ap()
    xi = nc.dram_tensor("xi", [NCHUNK, 128, F_IN], f32, kind="ExternalInput").ap()
    out = nc.dram_tensor("out", [NCHUNK, 128, F_OUT], f32, kind="ExternalOutput").ap()

    with tile.TileContext(nc) as tc:
        # Persistent output staging buffers.  Lanes {1,2,3,7} (mod 8) of the
        # output are always zero, so zero the buffers once and only write
        # lanes {0,4,5,6} each iteration.
        obuf, _free = tc.tile([128, NBUF_OUT * F_OUT], f32, name="obuf")
        for s in range(NBUF_OUT):
            nc.vector.memset(obuf[:, s * F_OUT : (s + 1) * F_OUT], 0.0)

        with (
            tc.tile_pool(name="io", bufs=3) as iop,
            tc.tile_pool(name="tmp", bufs=2) as tp,
        ):
            for i in range(NCHUNK):
                xt = iop.tile([128, F_IN], f32, tag="xt", name=f"xt{i}")
                yt = iop.tile([128, F_IN], f32, tag="yt", name=f"yt{i}")
                nc.sync.dma_start(out=xt, in_=xr[i])
                nc.sync.dma_start(out=yt, in_=xi[i])

                x0 = xt[:, 0::4]   # r00
                x1 = xt[:, 1::4]   # r01
                x2 = xt[:, 2::4]   # r10
                x3 = xt[:, 3::4]   # r11
                y1 = yt[:, 1::4]   # s01
                y2 = yt[:, 2::4]   # s10

                ob = obuf[:, (i % NBUF_OUT) * F_OUT : (i % NBUF_OUT + 1) * F_OUT]
                o0 = ob[:, 0::8]   # l11
                o4 = ob[:, 4::8]   # Re l21
                o5 = ob[:, 5::8]   # Im l21
                o6 = ob[:, 6::8]   # l22

                br = tp.tile([128, KC], f32, tag="br", name=f"br{i}")
                bi = tp.tile([128, KC], f32, tag="bi", name=f"bi{i}")
                a2 = tp.tile([128, KC], f32, tag="a2", name=f"a2{i}")
                ia = tp.tile([128, KC], f32, tag="ia", name=f"ia{i}")
                h = tp.tile([128, KC], f32, tag="h", name=f"h{i}")
                p = tp.tile([128, KC], f32, tag="p", name=f"p{i}")
                q = tp.tile([128, KC], f32, tag="q", name=f"q{i}")
                sm = tp.tile([128, KC], f32, tag="sm", name=f"sm{i}")
                g = tp.tile([128, KC], f32, tag="g", name=f"g{i}")

                # l11 = sqrt(r00 + 2)
                nc.scalar.activation(o0, x0, AF.Sqrt, bias=2.0)
                # br2 = r01 + r10 = 2*br ; bi2 = s10 - s01 = -2*bi
                nc.vector.tensor_add(br, x1, x2)
                nc.vector.tensor_sub(bi, y2, y1)
                # a2 = 2*r00 + 4 = 2a ; ia = 1/(2a) ; h = l11/(2a) = 0.5/sqrt(a)
                nc.vector.tensor_scalar(a2, x0, 2.0, 4.0, OP.mult, OP.add)
                nc.vector.reciprocal_approx_fast(ia, a2)
                nc.vector.tensor_mul(h, o0, ia)
                # l21 = (2br)*h - i*(2bi)*h
                nc.vector.tensor_mul(o4, br, h)
                nc.vector.tensor_mul(o5, bi, h)
                # |l21|^2 and l22 = sqrt(r11 - |l21|^2 + 2)
                nc.scalar.activation(p, o4, AF.Square)
                nc.scalar.activation(q, o5, AF.Square)
                nc.vector.tensor_add(sm, p, q)
                nc.vector.tensor_sub(g, x3, sm)
                nc.scalar.activation(o6, g, AF.Sqrt, bias=2.0)

                nc.gpsimd.dma_start(out=out[i], in_=ob)

    _CACHE["nc"] = nc
    return nc


def kernel(real_part, imag_part):
    nc = _build_nc()
    xr = np.ascontiguousarray(np.asarray(real_part), dtype=np.float32).reshape(
        NCORE, NCHUNK, 128, F_IN
    )
    xi = np.ascontiguousarray(np.asarray(imag_part), dtype=np.float32).reshape(
        NCORE, NCHUNK, 128, F_IN
    )
    in_maps = [{"xr": xr[c], "xi": xi[c]} for c in range(NCORE)]
    res = run_bass_kernel_spmd(nc, in_maps, core_ids=list(range(NCORE)))
    outs = [
        res.results[c]["out"].reshape(-1).view(np.complex64) for c in range(NCORE)
    ]
    return np.concatenate(outs).reshape(1, B, 2, 2)


# revision 10
# speedup vs baseline: 1.0064x; 1.0064x over previous
"""Batched 2x2 complex Hermitian Cholesky on 8 Trainium2 NeuronCores.

Math per matrix (inputs r = real_part[m], s = imag_part[m], both 2x2 f32):
    a   = r00 + 2                      (diag of Hermitian + M*I, M=2)
    c   = r11 + 2
    br  = (r01 + r10) / 2              (real part of A[0,1])
    bi  = (s01 - s10) / 2              (imag part of A[0,1])
    l11 = sqrt(a)
    l21 = conj(b) / l11 = (br - i*bi) / sqrt(a)
    l22 = sqrt(c - |l21|^2)
Output (complex64, upper triangle zero):
    [[l11, 0], [l21, l22]]

Layout: each matrix is 4 contiguous f32 in the input, 8 contiguous f32
(4 complex) in the output.  Per core we process NCHUNK chunks of
128 partitions x KC matrices; all HBM traffic is fully contiguous, the
de-interleave (stride 4/8) happens in SBUF via strided access patterns.
"""

import numpy as np

import concourse.bass as bass
import concourse.bacc as bacc
import concourse.mybir as mybir
from concourse import tile
from concourse.bass_utils import run_bass_kernel_spmd

B = 4194304
NCORE = 8
BC = B // NCORE            # 524288 matrices per core
KC = 512                   # matrices per partition per chunk
NCHUNK = BC // (128 * KC)  # 8
F_IN = 4 * KC              # f32 per partition per chunk (input tensors)
F_OUT = 8 * KC             # f32 per partition per chunk (output)
NBUF_OUT = 3               # rotation depth of persistent output buffers

_CACHE = {}


def _build_nc(nchunk=NCHUNK, kc=KC, reps=1):
    key = (nchunk, kc, reps)
    if key in _CACHE:
        return _CACHE[key]
    F_IN = 4 * kc
    F_OUT = 8 * kc
    f32 = mybir.dt.float32
    AF = mybir.ActivationFunctionType
    OP = mybir.AluOpType

    nc = bacc.Bacc("TRN2", target_bir_lowering=False, debug=False)
    # register a [128,1] constant 2.0 for activation bias (sqrt(x+2))
    c2 = nc.alloc_sbuf_tensor("const-float32-2.0", [128, 1], f32)
    nc.gpsimd.memset(c2.ap(), 2.0)
    nc.const_aps.aps[(f32, 2.0)] = c2.ap()
    nc.all_engine_barrier()

    xr = nc.dram_tensor("xr", [nchunk, 128, F_IN], f32, kind="ExternalInput").ap()
    xi = nc.dram_tensor("xi", [nchunk, 128, F_IN], f32, kind="ExternalInput").ap()
    out = nc.dram_tensor("out", [nchunk, 128, F_OUT], f32, kind="ExternalOutput").ap()

    with tile.TileContext(nc) as tc:
        # Persistent output staging buffers.  Lanes {1,2,3,7} (mod 8) of the
        # output are always zero, so zero the buffers once and only write
        # lanes {0,4,5,6} each iteration.
        obuf, _free = tc.tile([128, NBUF_OUT * F_OUT], f32, name="obuf")
        for s in range(NBUF_OUT):
            nc.vector.memset(obuf[:, s * F_OUT : (s + 1) * F_OUT], 0.0)

        # Warm up the ACT sqrt table set on a dummy input so the
        # PSEUDO_LOAD_ACT_FUNC_SET attaches to an instruction with no
        # sync waits (walrus can't encode table-load + 2 waits at once).
        warm, _freew = tc.tile([128, 1], f32, name="actwarm")
        nc.scalar.activation(warm, c2.ap(), AF.Sqrt, bias=2.0)
        _freew()

        with (
            tc.tile_pool(name="io", bufs=3) as iop,
            tc.tile_pool(name="tmp", bufs=2) as tp,
        ):
          def _body():
            for r in range(1):
              for i in range(nchunk):
                xt = iop.tile([128, F_IN], f32, tag="xt", name=f"xt{r}_{i}")
                yt = iop.tile([128, F_IN], f32, tag="yt", name=f"yt{r}_{i}")
                nc.sync.dma_start(out=xt, in_=xr[i])
                nc.sync.dma_start(out=yt, in_=xi[i])

                x0 = xt[:, 0::4]   # r00
                x1 = xt[:, 1::4]   # r01
                x2 = xt[:, 2::4]   # r10
                x3 = xt[:, 3::4]   # r11
                y1 = yt[:, 1::4]   # s01
                y2 = yt[:, 2::4]   # s10

                ob = obuf[:, (i % NBUF_OUT) * F_OUT : (i % NBUF_OUT + 1) * F_OUT]
                o0 = ob[:, 0::8]   # l11
                o4 = ob[:, 4::8]   # Re l21
                o5 = ob[:, 5::8]   # Im l21
                o6 = ob[:, 6::8]   # l22

                br = tp.tile([128, kc], f32, tag="br", name=f"br{r}_{i}")
                bi = tp.tile([128, kc], f32, tag="bi", name=f"bi{r}_{i}")
                a2 = tp.tile([128, kc], f32, tag="a2", name=f"a2{r}_{i}")
                ia = tp.tile([128, kc], f32, tag="ia", name=f"ia{r}_{i}")
                h = tp.tile([128, kc], f32, tag="h", name=f"h{r}_{i}")
                p = tp.tile([128, kc], f32, tag="p", name=f"p{r}_{i}")
                q = tp.tile([128, kc], f32, tag="q", name=f"q{r}_{i}")
                sm = tp.tile([128, kc], f32, tag="sm", name=f"sm{r}_{i}")
                g = tp.tile([128, kc], f32, tag="g", name=f"g{r}_{i}")

                # l11 = sqrt(r00 + 2)
                nc.scalar.activation(o0, x0, AF.Sqrt, bias=2.0)
                # br2 = r01 + r10 = 2*br ; bi2 = s10 - s01 = -2*bi
                nc.vector.tensor_add(br, x1, x2)
                nc.vector.tensor_sub(bi, y2, y1)
                # a2 = 2*r00 + 4 = 2a ; ia = 1/(2a) ; h = l11/(2a) = 0.5/sqrt(a)
                nc.vector.tensor_scalar(a2, x0, 2.0, 4.0, OP.mult, OP.add)
                nc.vector.reciprocal_approx_fast(ia, a2)
                nc.vector.tensor_mul(h, o0, ia)
                # l21 = (2br)*h - i*(2bi)*h
                nc.vector.tensor_mul(o4, br, h)
                nc.vector.tensor_mul(o5, bi, h)
                # |l21|^2 and l22 = sqrt(r11 - |l21|^2 + 2)
                nc.scalar.activation(p, o4, AF.Square)
                nc.scalar.activation(q, o5, AF.Square)
                nc.vector.tensor_add(sm, p, q)
                nc.vector.tensor_sub(g, x3, sm)
                nc.scalar.activation(o6, g, AF.Sqrt, bias=2.0)

                nc.gpsimd.dma_start(out=out[i], in_=ob)

          if reps == 1:
              _body()
          else:
              with tc.For_i(0, reps, 1):
                  _body()

        _free()

    nc.compile()
    _CACHE[key] = nc
    return nc


def kernel(real_part, imag_part):
    nc = _build_nc()
    xr = np.ascontiguousarray(np.asarray(real_part), dtype=np.float32).reshape(
        NCORE, NCHUNK, 128, F_IN
    )
    xi = np.ascontiguousarray(np.asarray(imag_part), dtype=np.float32).reshape(
        NCORE, NCHUNK, 128, F_IN
    )
    in_maps = [{"xr": xr[c], "xi": xi[c]} for c in range(NCORE)]
    res = run_bass_kernel_spmd(nc, in_maps, core_ids=list(range(NCORE)))
    outs = [
        res.results[c]["out"].reshape(-1).view(np.complex64) for c in range(NCORE)
    ]
    return np.concatenate(outs).reshape(1, B, 2, 2)
